# revision 19
# baseline (speedup 1.0000x reference)
"""Trainium2 Bass kernel: 2-layer LSTM (T=80, H=256) + embedding + softmax CE loss.

Strategy: data-parallel over batch (8192 -> 8 cores x 1024).  Everything runs
in a transposed layout: states/gates keep the hidden/gate dim on SBUF
partitions and the batch dim on the free axis, so the recurrent matmuls need
no per-step transposes (stationary = weights, moving = state).

The embedding lookup is reformulated as a one-hot matmul:
    x_t @ W1x  ==  onehot(feat_t) @ (emb @ W1x) = onehot @ E1
E1' = emb @ W1x + b1 + forget_bias_one_hot_fold, so layer-1 needs no bias adds
at all (each sample picks exactly one one-hot row).  The one-hot itself is an
is_equal compare against an iota column, with the feature row replicated
across 80 partitions host-side (pure layout prep).

A logical [256, 1024] tensor is stored "folded" as one SBUF tile [128, 2048]:
hidden unit u lives at (partition u % 128, col-block u // 128).  Gate g's
pre-activations accumulate in one PSUM tile [128, 2048] (4 banks): block ml
holds gate dims 256 g + 128 ml + p.

Final loss: logits computed as Wd.T @ h2 -> [80, B], PE-transposed back to
[B-chunk, 80] so log-sum-exp and the label gather run along the free axis.
"""

import sys

sys.path.insert(0, "/opt/trn_rl_repo")

import numpy as np

import concourse.bass as bass
import concourse.mybir as mybir
import concourse.tile as tile
from concourse import bacc
from concourse.bass_utils import run_bass_kernel_spmd

AF = mybir.ActivationFunctionType
OP = mybir.AluOpType
F32 = mybir.dt.float32
I32 = mybir.dt.int32

P = 128          # partitions
N_CORES = 8
B = 1024         # per-core batch shard
T = 80           # seq len
C = 80           # num classes
E = 8            # emb dim
H = 256          # hidden
G = 4 * H        # gates = 1024
NB = B // 512    # moving-operand chunks of 512 (fp32 max free dim)

GATE_FUNCS = [AF.Sigmoid, AF.Tanh, AF.Sigmoid, AF.Sigmoid]  # i, j, f, o


def build_program(T_steps=T):
    # Bacc (not plain Bass): its compile() runs generate_event_semaphores,
    # which splits excess per-instruction sync waits onto InstEventSemaphore
    # nops — walrus only allows one wait on LDWEIGHTS/MATMULT.
    nc = bacc.Bacc("TRN2", target_bir_lowering=False, debug=False,
                   enable_asserts=False, num_devices=N_CORES)

    # ---------------- DRAM I/O ----------------
    featrep = nc.dram_tensor("featrep", [T_steps, C, B], F32, kind="ExternalInput").ap()
    labelsT = nc.dram_tensor("labelsT", [P, B // P], F32, kind="ExternalInput").ap()
    emb = nc.dram_tensor("emb", [C, E], F32, kind="ExternalInput").ap()
    W1d = nc.dram_tensor("W1", [E + H, G], F32, kind="ExternalInput").ap()
    b1d = nc.dram_tensor("b1", [G], F32, kind="ExternalInput").ap()
    W2d = nc.dram_tensor("W2", [2 * H, G], F32, kind="ExternalInput").ap()
    b2d = nc.dram_tensor("b2", [G], F32, kind="ExternalInput").ap()
    Wdd = nc.dram_tensor("Wd", [H, C], F32, kind="ExternalInput").ap()
    bdd = nc.dram_tensor("bd", [C], F32, kind="ExternalInput").ap()
    ident = nc.dram_tensor("ident80", [C, C], F32, kind="ExternalInput").ap()
    iota80d = nc.dram_tensor("iota80", [C, 1], F32, kind="ExternalInput").ap()
    iotalabd = nc.dram_tensor("iotalab", [P, C], F32, kind="ExternalInput").ap()
    lossd = nc.dram_tensor("loss", [P, B // P], F32, kind="ExternalOutput").ap()

    with tile.TileContext(nc) as tc:
        _emit(nc, tc, featrep, labelsT, emb, W1d, b1d, W2d, b2d, Wdd, bdd,
              ident, iota80d, iotalabd, lossd, T_steps)
    nc.compile()
    return nc


def _emit(nc, tc, featrep, labelsT, emb, W1d, b1d, W2d, b2d, Wdd, bdd,
          ident, iota80d, iotalabd, lossd, T_steps=T):
    f32, i32 = F32, I32

    const = tc.alloc_tile_pool(name="const", bufs=1)

    # ---------------- resident weights/constants ----------------
    W1h = const.tile([P, 2 * G], f32)          # k-chunk k at cols [G k, G k + G)
    for k in range(2):
        nc.sync.dma_start(out=W1h[:, k * G:(k + 1) * G],
                          in_=W1d[E + P * k: E + P * (k + 1), :])
    W2 = const.tile([P, 4 * G], f32)
    for k in range(4):
        nc.sync.dma_start(out=W2[:, k * G:(k + 1) * G],
                          in_=W2d[P * k: P * (k + 1), :])
    Wd = const.tile([P, 2 * C], f32)
    for k in range(2):
        nc.sync.dma_start(out=Wd[:, k * C:(k + 1) * C],
                          in_=Wdd[P * k: P * (k + 1), :])
    b2c = const.tile([P, G // P], f32)          # col m = b2[128 m : 128 m + 128]
    nc.sync.dma_start(out=b2c, in_=b2d.rearrange("(m p) -> p m", p=P))
    bdc = const.tile([C, 1], f32)
    nc.sync.dma_start(out=bdc, in_=bdd[:, None])
    id80 = const.tile([C, C], f32)
    nc.sync.dma_start(out=id80, in_=ident)
    iota80 = const.tile([C, 1], f32)
    nc.sync.dma_start(out=iota80, in_=iota80d)
    iotalab = const.tile([P, C], f32)
    nc.sync.dma_start(out=iotalab, in_=iotalabd)
    labT = const.tile([P, B // P], f32)
    nc.sync.dma_start(out=labT, in_=labelsT)

    # forget-gate bias for layer 2: +1.0 on gate dims [512, 768) = cols 4,5
    nc.vector.tensor_scalar_add(b2c[:, 4:6], b2c[:, 4:6], 1.0)

    # ---------------- E1' = emb @ W1x + b1 (+1 on f-range) ----------------
    startup = tc.alloc_tile_pool(name="startup", bufs=1)
    embT = startup.tile([E, C], f32)
    nc.sync.dma_start(out=embT, in_=emb.rearrange("c e -> e c"))
    W1x = startup.tile([E, G], f32)
    nc.sync.dma_start(out=W1x, in_=W1d[0:E, :])
    b1row = startup.tile([1, G], f32)
    nc.sync.dma_start(out=b1row, in_=b1d[None, :])
    ones1 = startup.tile([1, C], f32)
    nc.vector.memset(ones1, 1.0)

    pstart = tc.alloc_tile_pool(name="pstart", bufs=1, space="PSUM")
    e1ps = pstart.tile([C, G], f32)
    for n in range(2):
        s = slice(512 * n, 512 * (n + 1))
        nc.tensor.matmul(e1ps[:, s], embT, W1x[:, s], start=True, stop=False)
        nc.tensor.matmul(e1ps[:, s], ones1, b1row[:, s], start=False, stop=True)
    E1 = const.tile([C, G], f32)
    nc.scalar.copy(E1, e1ps)
    nc.vector.tensor_scalar_add(E1[:, 512:768], E1[:, 512:768], 1.0)  # forget bias
    pstart.release()
    startup.release()

    # ---------------- pools for the recurrent loop ----------------
    states = tc.alloc_tile_pool(name="states", bufs=2)
    gates = tc.alloc_tile_pool(name="gates", bufs=1)
    pgate = tc.alloc_tile_pool(name="pgate", bufs=2, space="PSUM")
    feats = tc.alloc_tile_pool(name="feats", bufs=2)

    h1 = c1 = h2 = c2 = None

    for t in range(T_steps):
        featbc = feats.tile([C, B], f32, tag="featbc")
        nc.sync.dma_start(out=featbc, in_=featrep[t])
        oh = feats.tile([C, B], f32, tag="oh")
        nc.vector.tensor_scalar(oh, featbc, iota80[:, 0:1], None, op0=OP.is_equal)

        # ---- layer 1 gates ----
        sg1 = []
        for g in range(4):
            ps = pgate.tile([P, 2 * B], f32, tag="g", name=f"ps1_{t}_{g}")
            for ml in range(2):
                m = 2 * g + ml
                for n in range(NB):
                    dst = ps[:, ml * B + 512 * n: ml * B + 512 * (n + 1)]
                    rhs_oh = oh[:, 512 * n: 512 * (n + 1)]
                    nc.tensor.matmul(dst, E1[:, P * m: P * (m + 1)], rhs_oh,
                                     start=True, stop=(t == 0))
                    if t > 0:
                        for k in range(2):
                            nc.tensor.matmul(
                                dst,
                                W1h[:, G * k + P * m: G * k + P * (m + 1)],
                                h1[:, B * k + 512 * n: B * k + 512 * (n + 1)],
                                start=False, stop=(k == 1))
            sg = gates.tile([P, 2 * B], f32, tag=f"sg_{g}", name=f"sg1_{g}")
            nc.scalar.activation(sg, ps, GATE_FUNCS[g])
            sg1.append(sg)

        si, sj, sf, so = sg1
        # ---- layer 1 cell ----
        nc.vector.tensor_mul(si, si, sj)                  # si <- sigmoid(i)*tanh(j)
        c1n = states.tile([P, 2 * B], f32, tag="c1", name="c1")
        if t == 0:
            nc.vector.tensor_copy(c1n, si)
        else:
            nc.vector.tensor_mul(sf, c1, sf)              # sf <- c * sigmoid(f+1)
            nc.vector.tensor_add(c1n, sf, si)
        th1 = gates.tile([P, 2 * B], f32, tag="th", name="th1", bufs=2)
        nc.scalar.activation(th1, c1n, AF.Tanh)
        h1n = states.tile([P, 2 * B], f32, tag="h1", name="h1")
        nc.vector.tensor_mul(h1n, th1, so)
        c1, h1 = c1n, h1n

        # ---- layer 2 gates ----
        sg2 = []
        for g in range(4):
            ps = pgate.tile([P, 2 * B], f32, tag="g", name=f"ps2_{t}_{g}")
            for ml in range(2):
                m = 2 * g + ml
                for n in range(NB):
                    dst = ps[:, ml * B + 512 * n: ml * B + 512 * (n + 1)]
                    kmax = 2 if t == 0 else 4
                    for k in range(kmax):
                        src = h1n if k < 2 else h2
                        kk = k % 2
                        nc.tensor.matmul(
                            dst,
                            W2[:, G * k + P * m: G * k + P * (m + 1)],
                            src[:, B * kk + 512 * n: B * kk + 512 * (n + 1)],
                            start=(k == 0), stop=(k == kmax - 1))
            sg = gates.tile([P, 2 * B], f32, tag=f"sg_{g}", name=f"sg2_{g}")
            for ml in range(2):
                m = 2 * g + ml
                nc.scalar.activation(sg[:, ml * B:(ml + 1) * B],
                                     ps[:, ml * B:(ml + 1) * B],
                                     GATE_FUNCS[g], bias=b2c[:, m:m + 1])
            sg2.append(sg)

        si2, sj2, sf2, so2 = sg2
        nc.vector.tensor_mul(si2, si2, sj2)
        c2n = states.tile([P, 2 * B], f32, tag="c2", name="c2")
        if t == 0:
            nc.vector.tensor_copy(c2n, si2)
        else:
            nc.vector.tensor_mul(sf2, c2, sf2)
            nc.vector.tensor_add(c2n, sf2, si2)
        th2 = gates.tile([P, 2 * B], f32, tag="th", name="th2", bufs=2)
        nc.scalar.activation(th2, c2n, AF.Tanh)
        h2n = states.tile([P, 2 * B], f32, tag="h2", name="h2")
        nc.vector.tensor_mul(h2n, th2, so2)
        c2, h2 = c2n, h2n

    feats.release()
    pgate.release()

    # ---------------- loss ----------------
    ploss = tc.alloc_tile_pool(name="ploss", bufs=1, space="PSUM")
    lpool = tc.alloc_tile_pool(name="lpool", bufs=2)

    lps = ploss.tile([C, B], f32, tag="logits")
    for n in range(NB):
        for k in range(2):
            nc.tensor.matmul(
                lps[:, 512 * n: 512 * (n + 1)],
                Wd[:, C * k: C * (k + 1)],
                h2[:, B * k + 512 * n: B * k + 512 * (n + 1)],
                start=(k == 0), stop=(k == 1))
    logits = lpool.tile([C, B], f32, tag="logits_sb", bufs=1)
    nc.scalar.activation(logits, lps, AF.Identity, bias=bdc[:, 0:1])

    loss_sb = lpool.tile([P, B // P], f32, tag="loss_sb", bufs=1)
    sumexps, lablogs = [], []
    for cb in range(B // P):
        lt = ploss.tile([P, C], f32, tag="lt", bufs=2, name=f"lt_{cb}")
        nc.tensor.transpose(lt, logits[:, P * cb: P * (cb + 1)], id80)
        ohl = lpool.tile([P, C], f32, tag="ohl", name=f"ohl_{cb}")
        nc.vector.tensor_scalar(ohl, iotalab, labT[:, cb:cb + 1], None,
                                op0=OP.is_equal)
        lablog = lpool.tile([P, 1], f32, tag="lablog", name=f"lablog_{cb}")
        scr1 = lpool.tile([P, C], f32, tag="scr1", name=f"scr1_{cb}")
        nc.vector.scalar_tensor_tensor(scr1, lt, 1.0, ohl,
                                       op0=OP.mult, op1=OP.mult,
                                       accum_out=lablog)
        scr2 = lpool.tile([P, C], f32, tag="scr2", name=f"scr2_{cb}")
        sumexp = lpool.tile([P, 1], f32, tag="sumexp", name=f"sumexp_{cb}")
        nc.scalar.activation(scr2, lt, AF.Exp, accum_out=sumexp)
        sumexps.append(sumexp)
        lablogs.append(lablog)
    for cb in range(B // P):
        lse = lpool.tile([P, 1], f32, tag="lse", name=f"lse_{cb}")
        nc.scalar.activation(lse, sumexps[cb], AF.Ln)
        nc.vector.tensor_sub(loss_sb[:, cb:cb + 1], lse, lablogs[cb])

    nc.sync.dma_start(out=lossd, in_=loss_sb)
    lpool.release()
    ploss.release()
    gates.release()
    states.release()
    const.release()


# ---------------------------------------------------------------------------
# host side
# ---------------------------------------------------------------------------
_CACHE = {}


def _get_program():
    if "nc" not in _CACHE:
        _CACHE["nc"] = build_program()
    return _CACHE["nc"]


def make_in_maps(features, labels, embedding, W1, b1, W2, b2, Wd, bd):
    """Shard the full inputs into 8 per-core input maps."""
    features = np.asarray(features, dtype=np.int32)
    labels = np.asarray(labels, dtype=np.int32)
    shared = {
        "emb": np.ascontiguousarray(embedding, dtype=np.float32),
        "W1": np.ascontiguousarray(W1, dtype=np.float32),
        "b1": np.ascontiguousarray(b1, dtype=np.float32),
        "W2": np.ascontiguousarray(W2, dtype=np.float32),
        "b2": np.ascontiguousarray(b2, dtype=np.float32),
        "Wd": np.ascontiguousarray(Wd, dtype=np.float32),
        "bd": np.ascontiguousarray(bd, dtype=np.float32),
        "ident80": np.eye(C, dtype=np.float32),
        "iota80": np.arange(C, dtype=np.float32).reshape(C, 1),
        "iotalab": np.ascontiguousarray(
            np.broadcast_to(np.arange(C, dtype=np.float32)[None, :], (P, C))),
    }
    in_maps = []
    for c in range(N_CORES):
        fs = features[B * c: B * (c + 1)]            # [B, T]
        ls = labels[B * c: B * (c + 1)]              # [B]
        ft = fs.T                                    # [T, B]
        featrep = np.ascontiguousarray(
            np.broadcast_to(ft[:, None, :], (T, C, B)), dtype=np.float32)
        labT = np.ascontiguousarray(ls.reshape(B // P, P).T.astype(np.float32))  # [P, B//P]
        in_maps.append({**shared, "featrep": featrep, "labelsT": labT})
    return in_maps


def gather_output(results):
    outs = []
    for r in results:
        outs.append(np.asarray(r["loss"]).T.reshape(-1))   # [P, B//P] -> [B]
    return np.concatenate(outs, axis=0).astype(np.float32)


def kernel(features, labels, embedding, W1, b1, W2, b2, Wd, bd):
    nc = _get_program()
    in_maps = make_in_maps(features, labels, embedding, W1, b1, W2, b2, Wd, bd)
    res = run_bass_kernel_spmd(nc, in_maps, core_ids=list(range(N_CORES)))
    return gather_output(res.results)


# revision 26
# speedup vs baseline: 13.5387x; 13.5387x over previous
"""Trainium2 Bass kernel: 2-layer LSTM (T=80, H=256) + embedding + softmax CE loss.

Strategy: data-parallel over batch (8192 -> 8 cores x 1024).  Everything runs
in a transposed layout: states/gates keep the hidden/gate dim on SBUF
partitions and the batch dim on the free axis, so the recurrent matmuls need
no per-step transposes (stationary = weights, moving = state).

The embedding lookup is reformulated as a one-hot matmul:
    x_t @ W1x  ==  onehot(feat_t) @ (emb @ W1x) = onehot @ E1
E1' = emb @ W1x + b1 + forget_bias_one_hot_fold, so layer-1 needs no bias adds
at all (each sample picks exactly one one-hot row).  The one-hot itself is an
is_equal compare against an iota column, with the feature row replicated
across 80 partitions host-side (pure layout prep).

A logical [256, 1024] tensor is stored "folded" as one SBUF tile [128, 2048]:
hidden unit u lives at (partition u % 128, col-block u // 128).  Gate g's
pre-activations accumulate in one PSUM tile [128, 2048] (4 banks): block ml
holds gate dims 256 g + 128 ml + p.

Final loss: logits computed as Wd.T @ h2 -> [80, B], PE-transposed back to
[B-chunk, 80] so log-sum-exp and the label gather run along the free axis.
"""

import sys

sys.path.insert(0, "/opt/trn_rl_repo")

import numpy as np

import concourse.bass as bass
import concourse.mybir as mybir
import concourse.tile as tile
from concourse import bacc
from concourse.bass_utils import run_bass_kernel_spmd

AF = mybir.ActivationFunctionType
OP = mybir.AluOpType
F32 = mybir.dt.float32
BF16 = mybir.dt.bfloat16
I32 = mybir.dt.int32
DT = BF16          # dtype for weights / states / gate activations (matmul operands)

P = 128          # partitions
N_CORES = 8
B = 1024         # per-core batch shard
T = 80           # seq len
C = 80           # num classes
E = 8            # emb dim
H = 256          # hidden
G = 4 * H        # gates = 1024
NB = B // 512    # moving-operand chunks of 512 (fp32 max free dim)

GATE_FUNCS = [AF.Sigmoid, AF.Tanh, AF.Sigmoid, AF.Sigmoid]  # i, j, f, o


def build_program(T_steps=T, thin=None):
    # Bacc (not plain Bass): its compile() runs generate_event_semaphores,
    # which splits excess per-instruction sync waits onto InstEventSemaphore
    # nops — walrus only allows one wait on LDWEIGHTS/MATMULT.
    nc = bacc.Bacc("TRN2", target_bir_lowering=False, debug=False,
                   enable_asserts=False, num_devices=N_CORES)

    # ---------------- DRAM I/O ----------------
    featrep = nc.dram_tensor("featrep", [T_steps, C, B], DT, kind="ExternalInput").ap()
    labelsT = nc.dram_tensor("labelsT", [P, B // P], F32, kind="ExternalInput").ap()
    emb = nc.dram_tensor("emb", [C, E], DT, kind="ExternalInput").ap()
    W1d = nc.dram_tensor("W1", [E + H, G], DT, kind="ExternalInput").ap()
    b1d = nc.dram_tensor("b1", [G], DT, kind="ExternalInput").ap()
    W2d = nc.dram_tensor("W2", [2 * H, G], DT, kind="ExternalInput").ap()
    b2d = nc.dram_tensor("b2", [G], F32, kind="ExternalInput").ap()
    Wdd = nc.dram_tensor("Wd", [H, C], DT, kind="ExternalInput").ap()
    bdd = nc.dram_tensor("bd", [C], F32, kind="ExternalInput").ap()
    ident = nc.dram_tensor("ident80", [C, C], F32, kind="ExternalInput").ap()
    iota80d = nc.dram_tensor("iota80", [C, 1], F32, kind="ExternalInput").ap()
    iotalabd = nc.dram_tensor("iotalab", [P, C], F32, kind="ExternalInput").ap()
    lossd = nc.dram_tensor("loss", [P, B // P], F32, kind="ExternalOutput").ap()

    with tile.TileContext(nc) as tc:
        _emit(nc, tc, featrep, labelsT, emb, W1d, b1d, W2d, b2d, Wdd, bdd,
              ident, iota80d, iotalabd, lossd, T_steps, thin)
    nc.compile()
    return nc


def _emit(nc, tc, featrep, labelsT, emb, W1d, b1d, W2d, b2d, Wdd, bdd,
          ident, iota80d, iotalabd, lossd, T_steps=T, thin=None):
    f32, i32 = F32, I32

    def act(out, in_, func, **kw):
        if thin == "act":
            nc.scalar.activation(out[:, 0:32], in_[:, 0:32], func, **kw)
        else:
            nc.scalar.activation(out, in_, func, **kw)

    def tt(out, a, b_, op):
        if thin == "dve":
            nc.vector.tensor_tensor(out[:, 0:32], a[:, 0:32], b_[:, 0:32], op=op)
        else:
            nc.vector.tensor_tensor(out, a, b_, op=op)

    const = tc.alloc_tile_pool(name="const", bufs=1)

    # ---------------- resident weights/constants ----------------
    W1h = const.tile([P, 2 * G], DT)          # k-chunk k at cols [G k, G k + G)
    for k in range(2):
        nc.sync.dma_start(out=W1h[:, k * G:(k + 1) * G],
                          in_=W1d[E + P * k: E + P * (k + 1), :])
    W2 = const.tile([P, 4 * G], DT)
    for k in range(4):
        nc.sync.dma_start(out=W2[:, k * G:(k + 1) * G],
                          in_=W2d[P * k: P * (k + 1), :])
    Wd = const.tile([P, 2 * C], DT)
    for k in range(2):
        nc.sync.dma_start(out=Wd[:, k * C:(k + 1) * C],
                          in_=Wdd[P * k: P * (k + 1), :])
    b2c = const.tile([P, G // P], f32)          # col m = b2[128 m : 128 m + 128]
    nc.sync.dma_start(out=b2c, in_=b2d.rearrange("(m p) -> p m", p=P))
    bdc = const.tile([C, 1], f32)
    nc.sync.dma_start(out=bdc, in_=bdd[:, None])
    id80 = const.tile([C, C], f32)
    nc.sync.dma_start(out=id80, in_=ident)
    iota80 = const.tile([C, 1], f32)
    nc.sync.dma_start(out=iota80, in_=iota80d)
    iotalab = const.tile([P, C], f32)
    nc.sync.dma_start(out=iotalab, in_=iotalabd)
    labT = const.tile([P, B // P], f32)
    nc.sync.dma_start(out=labT, in_=labelsT)

    # forget-gate bias for layer 2: +1.0 on gate dims [512, 768) = cols 4,5
    nc.vector.tensor_scalar_add(b2c[:, 4:6], b2c[:, 4:6], 1.0)

    # ---------------- E1' = emb @ W1x + b1 (+1 on f-range) ----------------
    startup = tc.alloc_tile_pool(name="startup", bufs=1)
    embT = startup.tile([E, C], DT)
    nc.sync.dma_start(out=embT, in_=emb.rearrange("c e -> e c"))
    W1x = startup.tile([E, G], DT)
    nc.sync.dma_start(out=W1x, in_=W1d[0:E, :])
    b1row = startup.tile([1, G], DT)
    nc.sync.dma_start(out=b1row, in_=b1d[None, :])
    ones1 = startup.tile([1, C], DT)
    nc.vector.memset(ones1, 1.0)

    pstart = tc.alloc_tile_pool(name="pstart", bufs=1, space="PSUM")
    e1ps = pstart.tile([C, G], f32)
    for n in range(2):
        s = slice(512 * n, 512 * (n + 1))
        nc.tensor.matmul(e1ps[:, s], embT, W1x[:, s], start=True, stop=False)
        nc.tensor.matmul(e1ps[:, s], ones1, b1row[:, s], start=False, stop=True)
    E1 = const.tile([C, G], DT)
    nc.scalar.copy(E1, e1ps)
    nc.vector.tensor_scalar_add(E1[:, 512:768], E1[:, 512:768], 1.0)  # forget bias
    pstart.release()
    startup.release()

    # ---------------- pools for the recurrent loop ----------------
    states = tc.alloc_tile_pool(name="states", bufs=2)
    gates = tc.alloc_tile_pool(name="gates", bufs=2)
    pgate = tc.alloc_tile_pool(name="pgate", bufs=4, space="PSUM")
    feats = tc.alloc_tile_pool(name="feats", bufs=3)

    h1 = c1 = h2 = c2 = None

    def l1_block(t, oh, h1_in):
        sg1 = []
        for g in range(4):
            sg = gates.tile([P, 2 * B], DT, tag=f"sg_{g}", name=f"sg1_{g}")
            for ml in range(2):
                m = 2 * g + ml
                ps = pgate.tile([P, B], f32, tag="g", name=f"ps1_{t}_{g}_{ml}")
                for n in range(NB):
                    dst = ps[:, 512 * n: 512 * (n + 1)]
                    rhs_oh = oh[:, 512 * n: 512 * (n + 1)]
                    nc.tensor.matmul(dst, E1[:, P * m: P * (m + 1)], rhs_oh,
                                     start=True, stop=(t == 0))
                    if t > 0:
                        for k in range(1 if thin == "pe" else 2):
                            nc.tensor.matmul(
                                dst,
                                W1h[:, G * k + P * m: G * k + P * (m + 1)],
                                h1_in[:, B * k + 512 * n: B * k + 512 * (n + 1)],
                                start=False, stop=(k == 1))
                act(sg[:, ml * B:(ml + 1) * B], ps, GATE_FUNCS[g])
            sg1.append(sg)
        return sg1

    def l2_block(t, h1_in, h2_in):
        sg2 = []
        for g in range(4):
            sg = gates.tile([P, 2 * B], DT, tag=f"sg_{g}", name=f"sg2_{g}")
            for ml in range(2):
                m = 2 * g + ml
                ps = pgate.tile([P, B], f32, tag="g", name=f"ps2_{t}_{g}_{ml}")
                for n in range(NB):
                    dst = ps[:, 512 * n: 512 * (n + 1)]
                    korder = [0, 1] if t == 0 else [0, 1, 2, 3]
                    if thin == "pe":
                        korder = korder[:1] if t == 0 else korder[:2]
                    for ki, k in enumerate(korder):
                        hsrc = h1_in if k < 2 else h2_in
                        kk = k % 2
                        nc.tensor.matmul(
                            dst,
                            W2[:, G * k + P * m: G * k + P * (m + 1)],
                            hsrc[:, B * kk + 512 * n: B * kk + 512 * (n + 1)],
                            start=(ki == 0), stop=(ki == len(korder) - 1))
                act(sg[:, ml * B:(ml + 1) * B], ps,
                    GATE_FUNCS[g], bias=b2c[:, m:m + 1])
            sg2.append(sg)
        return sg2

    def cell(t, sgates, c_in, ctag, htag, thname):
        si, sj, sf, so = sgates
        tt(si, si, sj, OP.mult)                  # si <- sigmoid(i)*tanh(j)
        cn = states.tile([P, 2 * B], DT, tag=ctag, name=ctag)
        if t == 0:
            nc.vector.tensor_copy(cn, si)
        else:
            tt(sf, c_in, sf, OP.mult)            # sf <- c * sigmoid(f+1)
            tt(cn, sf, si, OP.add)
        th = gates.tile([P, 2 * B], DT, tag="th", name=thname, bufs=2)
        act(th, cn, AF.Tanh)
        hn = states.tile([P, 2 * B], DT, tag=htag, name=htag)
        tt(hn, th, so, OP.mult)
        return cn, hn

    # Software pipeline: L2 runs one step behind L1, so every matmul's
    # inputs are ready when the PE reaches it (no PE stalls on the
    # recurrent chain).
    sg2_pend = None
    for t in range(T_steps):
        featbc = feats.tile([C, B], DT, tag="featbc")
        nc.sync.dma_start(out=featbc, in_=featrep[t])
        oh = feats.tile([C, B], DT, tag="oh")
        nc.vector.tensor_scalar(oh, featbc, iota80[:, 0:1], None, op0=OP.is_equal)

        h1_prev = h1
        sg1 = l1_block(t, oh, h1_prev)
        if t > 0:
            sg2_pend = l2_block(t - 1, h1_prev, h2)
        c1, h1 = cell(t, sg1, c1, "c1", "h1", "th1")
        if t > 0:
            c2, h2 = cell(t - 1, sg2_pend, c2, "c2", "h2", "th2")

    # drain the pipeline: L2 for the final step
    sg2_pend = l2_block(T_steps - 1, h1, h2)
    c2, h2 = cell(T_steps - 1, sg2_pend, c2, "c2", "h2", "th2")

    feats.release()
    pgate.release()

    # ---------------- loss ----------------
    ploss = tc.alloc_tile_pool(name="ploss", bufs=1, space="PSUM")
    lpool = tc.alloc_tile_pool(name="lpool", bufs=2)

    lps = ploss.tile([C, B], f32, tag="logits")
    for n in range(NB):
        for k in range(2):
            nc.tensor.matmul(
                lps[:, 512 * n: 512 * (n + 1)],
                Wd[:, C * k: C * (k + 1)],
                h2[:, B * k + 512 * n: B * k + 512 * (n + 1)],
                start=(k == 0), stop=(k == 1))
    logits = lpool.tile([C, B], f32, tag="logits_sb", bufs=1)
    nc.scalar.activation(logits, lps, AF.Identity, bias=bdc[:, 0:1])

    loss_sb = lpool.tile([P, B // P], f32, tag="loss_sb", bufs=1)
    # per-chunk sum-exps / label logits gathered as COLUMNS of shared tiles so
    # the log and the final subtract are single ops (one Exp->Ln table switch)
    sumexp_all = lpool.tile([P, B // P], f32, tag="sumexp_all", bufs=1)
    lablog_all = lpool.tile([P, B // P], f32, tag="lablog_all", bufs=1)
    for cb in range(B // P):
        lt = ploss.tile([P, C], f32, tag="lt", bufs=2, name=f"lt_{cb}")
        nc.tensor.transpose(lt, logits[:, P * cb: P * (cb + 1)], id80)
        ohl = lpool.tile([P, C], f32, tag="ohl", name=f"ohl_{cb}")
        nc.vector.tensor_scalar(ohl, iotalab, labT[:, cb:cb + 1], None,
                                op0=OP.is_equal)
        scr1 = lpool.tile([P, C], f32, tag="scr1", name=f"scr1_{cb}")
        nc.vector.scalar_tensor_tensor(scr1, lt, 1.0, ohl,
                                       op0=OP.mult, op1=OP.mult,
                                       accum_out=lablog_all[:, cb:cb + 1])
        scr2 = lpool.tile([P, C], f32, tag="scr2", name=f"scr2_{cb}")
        nc.scalar.activation(scr2, lt, AF.Exp,
                             accum_out=sumexp_all[:, cb:cb + 1])
    lse = lpool.tile([P, B // P], f32, tag="lse", bufs=1)
    nc.scalar.activation(lse, sumexp_all, AF.Ln)
    nc.vector.tensor_sub(loss_sb, lse, lablog_all)
    nc.sync.dma_start(out=lossd, in_=loss_sb)
    lpool.release()
    ploss.release()
    gates.release()
    states.release()
    const.release()


# ---------------------------------------------------------------------------
# host side
# ---------------------------------------------------------------------------
_CACHE = {}


def _get_program():
    if "nc" not in _CACHE:
        _CACHE["nc"] = build_program()
    return _CACHE["nc"]


def make_in_maps(features, labels, embedding, W1, b1, W2, b2, Wd, bd):
    """Shard the full inputs into 8 per-core input maps."""
    features = np.asarray(features, dtype=np.int32)
    labels = np.asarray(labels, dtype=np.int32)
    import ml_dtypes
    wdt = ml_dtypes.bfloat16 if DT == BF16 else np.float32
    shared = {
        "emb": np.ascontiguousarray(np.asarray(embedding, np.float32).astype(wdt)),
        "W1": np.ascontiguousarray(np.asarray(W1, np.float32).astype(wdt)),
        "b1": np.ascontiguousarray(np.asarray(b1, np.float32).astype(wdt)),
        "W2": np.ascontiguousarray(np.asarray(W2, np.float32).astype(wdt)),
        "b2": np.ascontiguousarray(b2, dtype=np.float32),
        "Wd": np.ascontiguousarray(np.asarray(Wd, np.float32).astype(wdt)),
        "bd": np.ascontiguousarray(bd, dtype=np.float32),
        "ident80": np.eye(C, dtype=np.float32),
        "iota80": np.arange(C, dtype=np.float32).reshape(C, 1),
        "iotalab": np.ascontiguousarray(
            np.broadcast_to(np.arange(C, dtype=np.float32)[None, :], (P, C))),
    }
    in_maps = []
    for c in range(N_CORES):
        fs = features[B * c: B * (c + 1)]            # [B, T]
        ls = labels[B * c: B * (c + 1)]              # [B]
        ft = fs.T                                    # [T, B]
        featrep = np.ascontiguousarray(
            np.broadcast_to(ft[:, None, :], (T, C, B))).astype(wdt)
        labT = np.ascontiguousarray(ls.reshape(B // P, P).T.astype(np.float32))  # [P, B//P]
        in_maps.append({**shared, "featrep": featrep, "labelsT": labT})
    return in_maps


def gather_output(results):
    outs = []
    for r in results:
        outs.append(np.asarray(r["loss"]).T.reshape(-1))   # [P, B//P] -> [B]
    return np.concatenate(outs, axis=0).astype(np.float32)


def kernel(features, labels, embedding, W1, b1, W2, b2, Wd, bd):
    nc = _get_program()
    in_maps = make_in_maps(features, labels, embedding, W1, b1, W2, b2, Wd, bd)
    res = run_bass_kernel_spmd(nc, in_maps, core_ids=list(range(N_CORES)))
    return gather_output(res.results)
